# revision 1
# baseline (speedup 1.0000x reference)
"""Trainium2 Bass kernel for SinusoidalEncoder.

Reference computation (per element):
  out[b, s, 2i]   = sin(x[b, s, 0] * f_i)
  out[b, s, 2i+1] = cos(x[b, s, 1] * f_i) = sin(x[b, s, 1] * f_i + pi/2)
  f_i = 2^(2i/256 * 10), i = 0..127

Strategy: pure data-parallel over batch (4 batches/core on 8 cores).
Per core: 32768 tokens. Tokens are laid out p-major: partition p handles
tokens p*256 + jc for jc in 0..255, so both the x load and the output
stores are large contiguous-per-partition DMAs.

Math: work in "turns": u = x * (f_i / 2pi)  (plus 1/4 turn for cos slots).
Range-reduce with the round-to-nearest magic constant M = 1.5 * 2^23:
  k  = fl(u + M) - M        (nearest integer to u)
  r  = u - k                in [-0.5, 0.5]
  out = sin(2*pi*r)         via ACT Sin (accurate for |arg| <= pi)

Engine assignment (all big dense ops, chunked at [128, 2048]):
  DVE:    per-token-group tensor_scalar phase multiplies + TT subtract
  ACT:    Identity (magic add via bias) + Sin (scale=2pi)
  GPSIMD: tensor_scalar (-M) to extract k
  HWDGE:  1MB output stores
"""
import numpy as np

_NCORES = 8
_B, _S, _C = 32, 8192, 3
_D = 256
_HALF = 128
_NTOK = (_B // _NCORES) * _S          # 32768 tokens per core
_JC = _NTOK // 128                    # 256 token-groups (tokens per partition)
_K = 8                                # token-groups per chunk
_NCHUNK = _JC // _K                   # 32 chunks
_CW = _D * _K                         # chunk width: 2048 floats
_MAGIC = 12582912.0                   # 1.5 * 2^23
_TWO_PI = float(2 * np.pi)

_cache = {}


def _freqs_half():
    # f_i / (2*pi) computed in f64 then rounded once to f32
    e = np.arange(0, _D, 2, dtype=np.float64) / _D * 10.0
    f = np.power(2.0, e)
    return (f / (2 * np.pi)).astype(np.float32)  # [128]


def _build():
    import concourse.bacc as bacc
    import concourse.tile as tile
    import concourse.mybir as mybir

    F32 = mybir.dt.float32
    AF = mybir.ActivationFunctionType
    ALU = mybir.AluOpType

    nc = bacc.Bacc("TRN2", target_bir_lowering=False, debug=False)
    x_d = nc.dram_tensor("x", [_NTOK, _C], F32, kind="ExternalInput")
    fh_d = nc.dram_tensor("fh", [128, _HALF], F32, kind="ExternalInput")
    out_d = nc.dram_tensor("out", [_NTOK, _D], F32, kind="ExternalOutput")

    x_view = x_d.ap().rearrange("(p f) c -> p (f c)", p=128)      # [128, 768]
    out_view = out_d.ap().rearrange("(p n) d -> p (n d)", p=128)  # [128, 65536]

    with tile.TileContext(nc) as tc:
        with (
            tc.tile_pool(name="const", bufs=1) as cpool,
            tc.tile_pool(name="u", bufs=2) as upool,
            tc.tile_pool(name="um", bufs=2) as umpool,
            tc.tile_pool(name="k", bufs=2) as kpool,
            tc.tile_pool(name="r", bufs=2) as rpool,
            tc.tile_pool(name="o", bufs=3) as opool,
        ):
            MB = cpool.tile([128, 1], F32, tag="MB")
            nc.gpsimd.memset(MB[:], _MAGIC)
            QB = cpool.tile([128, 1], F32, tag="QB")
            nc.gpsimd.memset(QB[:], 0.25)  # quarter turn for cos slots
            X = cpool.tile([128, 3 * _JC], F32, tag="X")
            nc.sync.dma_start(X[:], x_view)
            FH = cpool.tile([128, _HALF], F32, tag="FH")
            nc.sync.dma_start(FH[:], fh_d.ap())

            for c in range(_NCHUNK):
                # 1. phase multiplies (DVE): planar U stripes [even | odd]
                U = upool.tile([128, _CW], F32, tag="U")
                for jj in range(_K):
                    j = c * _K + jj
                    # even slots: u = f2 * x0
                    nc.vector.tensor_scalar(
                        U[:, jj * _D : jj * _D + _HALF],
                        FH[:],
                        X[:, 3 * j : 3 * j + 1],
                        None,
                        ALU.mult,
                    )
                    # odd slots: u = f2 * x1 + 1/4  (cos = sin + quarter turn)
                    nc.vector.tensor_scalar(
                        U[:, jj * _D + _HALF : (jj + 1) * _D],
                        FH[:],
                        X[:, 3 * j + 1 : 3 * j + 2],
                        QB[:, 0:1],
                        ALU.mult,
                        ALU.add,
                    )

                # 2. magic add (ACT): um = fl(u + M)  == k + M
                UM = umpool.tile([128, _CW], F32, tag="UM")
                nc.scalar.activation(UM[:], U[:], AF.Identity, bias=MB[:, 0:1])

                # 3. k extraction (GPSIMD): k = um - M
                K2 = kpool.tile([128, _CW], F32, tag="K2")
                nc.gpsimd.tensor_scalar(K2[:], UM[:], _MAGIC, None, ALU.subtract)

                # 4. subtract (DVE), writing interleaved: r[2i+b] = u[b*128+i] - k
                R = rpool.tile([128, _CW], F32, tag="R")
                r_int = R[:].rearrange("p (jj i b) -> p jj b i", jj=_K, b=2)
                u_pl = U[:].rearrange("p (jj b i) -> p jj b i", jj=_K, b=2)
                k_pl = K2[:].rearrange("p (jj b i) -> p jj b i", jj=_K, b=2)
                nc.vector.tensor_tensor(r_int, u_pl, k_pl, ALU.subtract)

                # 5. sin (ACT, dense): out = sin(2*pi*r)
                O = opool.tile([128, _CW], F32, tag="O")
                nc.scalar.activation(O[:], R[:], AF.Sin, scale=_TWO_PI)

                # 6. store (HWDGE): 1MB, 8KB contiguous per partition
                nc.sync.dma_start(out_view[:, c * _CW : (c + 1) * _CW], O[:])

    nc.compile()
    return nc


def _get_nc():
    if "nc" not in _cache:
        _cache["nc"] = _build()
    return _cache["nc"]


def kernel(x: np.ndarray) -> np.ndarray:
    from concourse.bass_utils import run_bass_kernel_spmd

    nc = _get_nc()
    x = np.ascontiguousarray(np.asarray(x, dtype=np.float32))
    fh = np.tile(_freqs_half()[None, :], (128, 1))
    shards = x.reshape(_NCORES, _NTOK, _C)
    in_maps = [{"x": shards[i], "fh": fh} for i in range(_NCORES)]
    res = run_bass_kernel_spmd(nc, in_maps, list(range(_NCORES)))
    out = np.stack([res.results[i]["out"] for i in range(_NCORES)])
    return out.reshape(_B, _S, _D)


# revision 3
# speedup vs baseline: 1.5619x; 1.5619x over previous
"""Trainium2 Bass kernel for SinusoidalEncoder.

Reference (per element):
  out[b, s, 2i]   = sin(x[b, s, 0] * f_i)
  out[b, s, 2i+1] = cos(x[b, s, 1] * f_i),  f_i = 2^(2i/256 * 10)

Sharding: pure data-parallel over batch (4 batches per core on 8 cores).
Per core 32768 tokens, laid out p-major (partition p owns tokens
p*256 + jc), so x loads and output stores are large per-partition
contiguous DMAs.

Math (in "turns"): u = x * (f_i/2pi), +1/4 turn on cos slots.
Magic-constant range reduction, M = 1.5*2^23:
  k  = fl(u + M) - M          (integer nearest u)
  -r = k - u  in [-0.5, 0.5]
  out = sin(-2pi * -r)

Per chunk [128, 2048] (8 token-groups x 256 interleaved slots):
  1. DVE  tensor_tensor (stride-0 broadcast APs):
       U[p, jj, i, b] = F2INT[p, i, b] * X[p, jj, b]     (one op)
  2. ACT  Identity strided over odd slots: U_odd += 0.25  (in place)
  3. ACT  Identity: UM = fl(U + M)
  4. DVE  scalar_tensor_tensor: R = (UM - M) - U  = -r    (one op)
  5. ACT  Sin: O = sin(R * -2pi)
  6. HWDGE store (1MB, 8KB/partition contiguous)
"""
import numpy as np

_NCORES = 8
_B, _S, _C = 32, 8192, 3
_D = 256
_NTOK = (_B // _NCORES) * _S          # 32768 tokens per core
_JC = _NTOK // 128                    # 256 token-groups per core
_K = 8                                # token-groups per chunk
_NCHUNK = _JC // _K                   # 32 chunks
_CW = _D * _K                         # 2048 floats per chunk
_MAGIC = 12582912.0                   # 1.5 * 2^23
_TWO_PI = float(2 * np.pi)

_cache = {}


def _freqs_half():
    e = np.arange(0, _D, 2, dtype=np.float64) / _D * 10.0
    return (np.power(2.0, e) / (2 * np.pi)).astype(np.float32)  # [128]


def _build():
    import concourse.bacc as bacc
    import concourse.tile as tile
    import concourse.mybir as mybir

    F32 = mybir.dt.float32
    AF = mybir.ActivationFunctionType
    ALU = mybir.AluOpType

    nc = bacc.Bacc("TRN2", target_bir_lowering=False, debug=False)
    x_d = nc.dram_tensor("x", [_NTOK, _C], F32, kind="ExternalInput")
    f2_d = nc.dram_tensor("f2i", [128, _D], F32, kind="ExternalInput")
    out_d = nc.dram_tensor("out", [_NTOK, _D], F32, kind="ExternalOutput")

    x_view = x_d.ap().rearrange("(p f) c -> p (f c)", p=128)      # [128, 768]
    out_view = out_d.ap().rearrange("(p n) d -> p (n d)", p=128)  # [128, 65536]

    with tile.TileContext(nc) as tc:
        with (
            tc.tile_pool(name="const", bufs=1) as cpool,
            tc.tile_pool(name="u", bufs=3) as upool,
            tc.tile_pool(name="um", bufs=2) as umpool,
            tc.tile_pool(name="r", bufs=2) as rpool,
            tc.tile_pool(name="o", bufs=3) as opool,
        ):
            MB = cpool.tile([128, 1], F32, tag="MB")
            nc.gpsimd.memset(MB[:], _MAGIC)
            QB = cpool.tile([128, 1], F32, tag="QB")
            nc.gpsimd.memset(QB[:], 0.25)
            X = cpool.tile([128, 3 * _JC], F32, tag="X")
            nc.sync.dma_start(X[:], x_view)
            F2 = cpool.tile([128, _D], F32, tag="F2")
            nc.sync.dma_start(F2[:], f2_d.ap())

            f2_b = (
                F2[:]
                .rearrange("p (i b) -> p i b", b=2)
                .unsqueeze(1)
                .broadcast_to((128, _K, 128, 2))
            )

            for c in range(_NCHUNK):
                # 1. phase: U[p, jj, i, b] = f2_i * x_b(token jj)
                U = upool.tile([128, _CW], F32, tag="U")
                u4 = U[:].rearrange("p (jj i b) -> p jj i b", jj=_K, b=2)
                x_b = (
                    X[:, 24 * c : 24 * c + 24]
                    .rearrange("p (jj b) -> p jj b", b=3)[:, :, 0:2]
                    .unsqueeze(2)
                    .broadcast_to((128, _K, 128, 2))
                )
                nc.vector.tensor_tensor(u4, f2_b, x_b, ALU.mult)

                # 2. quarter turn on odd (cos) slots, in place
                u_odd = u4[:, :, :, 1:2]
                nc.scalar.activation(u_odd, u_odd, AF.Identity, bias=QB[:, 0:1])

                # 3. magic add: UM = fl(U + M) = k + M
                UM = umpool.tile([128, _CW], F32, tag="UM")
                nc.scalar.activation(UM[:], U[:], AF.Identity, bias=MB[:, 0:1])

                # 4. fused: R = (UM - M) - U = k - u = -r
                R = rpool.tile([128, _CW], F32, tag="R")
                nc.vector.scalar_tensor_tensor(
                    R[:], UM[:], _MAGIC, U[:], ALU.subtract, ALU.subtract
                )

                # 5. sin: O = sin(-2pi * R) = sin(2pi r)
                O = opool.tile([128, _CW], F32, tag="O")
                nc.scalar.activation(O[:], R[:], AF.Sin, scale=-_TWO_PI)

                # 6. store
                nc.sync.dma_start(out_view[:, c * _CW : (c + 1) * _CW], O[:])

    nc.compile()
    return nc


def _get_nc():
    if "nc" not in _cache:
        _cache["nc"] = _build()
    return _cache["nc"]


def _f2i_input():
    f2 = _freqs_half()
    f2i = np.empty(_D, np.float32)
    f2i[0::2] = f2
    f2i[1::2] = f2
    return np.tile(f2i[None, :], (128, 1))


def kernel(x: np.ndarray) -> np.ndarray:
    from concourse.bass_utils import run_bass_kernel_spmd

    nc = _get_nc()
    x = np.ascontiguousarray(np.asarray(x, dtype=np.float32))
    f2i = _f2i_input()
    shards = x.reshape(_NCORES, _NTOK, _C)
    in_maps = [{"x": shards[i], "f2i": f2i} for i in range(_NCORES)]
    res = run_bass_kernel_spmd(nc, in_maps, list(range(_NCORES)))
    out = np.stack([res.results[i]["out"] for i in range(_NCORES)])
    return out.reshape(_B, _S, _D)


# revision 4
# speedup vs baseline: 1.6187x; 1.0364x over previous
"""Trainium2 Bass kernel for SinusoidalEncoder.

Reference (per element):
  out[b, s, 2i]   = sin(x[b, s, 0] * f_i)
  out[b, s, 2i+1] = cos(x[b, s, 1] * f_i),  f_i = 2^(2i/256 * 10)

Sharding: pure data-parallel over batch (4 batches per core on 8 cores).
Per core 32768 tokens, laid out p-major (partition p owns tokens
p*256 + jc), so x loads and output stores are large per-partition
contiguous DMAs.

Math (in "turns"): u = x * (f_i/2pi), +1/4 turn on cos slots.
Magic-constant range reduction, M = 1.5*2^23:
  k  = fl(u + M) - M          (integer nearest u)
  -r = k - u  in [-0.5, 0.5]
  out = sin(-2pi * -r)

Per chunk [128, 2048] (8 token-groups x 256 interleaved slots):
  1. DVE  tensor_tensor (stride-0 broadcast APs):
       U[p, jj, i, b] = F2INT[p, i, b] * X[p, jj, b]     (one op)
  2. ACT  Identity strided over odd slots: U_odd += 0.25  (in place)
  3. ACT  Identity: UM = fl(U + M)
  4. DVE  scalar_tensor_tensor: R = (UM - M) - U  = -r    (one op)
  5. ACT  Sin: O = sin(R * -2pi)
  6. HWDGE store (1MB, 8KB/partition contiguous)
"""
import numpy as np

_NCORES = 8
_B, _S, _C = 32, 8192, 3
_D = 256
_NTOK = (_B // _NCORES) * _S          # 32768 tokens per core
_JC = _NTOK // 128                    # 256 token-groups per core
_K = 16                               # token-groups per chunk
_NCHUNK = _JC // _K                   # 32 chunks
_CW = _D * _K                         # 2048 floats per chunk
_MAGIC = 12582912.0                   # 1.5 * 2^23
_TWO_PI = float(2 * np.pi)

_cache = {}


def _freqs_half():
    e = np.arange(0, _D, 2, dtype=np.float64) / _D * 10.0
    return (np.power(2.0, e) / (2 * np.pi)).astype(np.float32)  # [128]


def _build():
    import concourse.bacc as bacc
    import concourse.tile as tile
    import concourse.mybir as mybir

    F32 = mybir.dt.float32
    AF = mybir.ActivationFunctionType
    ALU = mybir.AluOpType

    nc = bacc.Bacc("TRN2", target_bir_lowering=False, debug=False)
    x_d = nc.dram_tensor("x", [_NTOK, _C], F32, kind="ExternalInput")
    f2_d = nc.dram_tensor("f2i", [128, _D], F32, kind="ExternalInput")
    out_d = nc.dram_tensor("out", [_NTOK, _D], F32, kind="ExternalOutput")

    x_view = x_d.ap().rearrange("(p f) c -> p (f c)", p=128)      # [128, 768]
    out_view = out_d.ap().rearrange("(p n) d -> p (n d)", p=128)  # [128, 65536]

    with tile.TileContext(nc) as tc:
        with (
            tc.tile_pool(name="const", bufs=1) as cpool,
            tc.tile_pool(name="u", bufs=3) as upool,
            tc.tile_pool(name="um", bufs=2) as umpool,
            tc.tile_pool(name="r", bufs=2) as rpool,
            tc.tile_pool(name="o", bufs=3) as opool,
        ):
            MB = cpool.tile([128, 1], F32, tag="MB")
            nc.gpsimd.memset(MB[:], _MAGIC)
            QB = cpool.tile([128, 1], F32, tag="QB")
            nc.gpsimd.memset(QB[:], 0.25)
            X = cpool.tile([128, 3 * _JC], F32, tag="X")
            nc.sync.dma_start(X[:], x_view)
            F2 = cpool.tile([128, _D], F32, tag="F2")
            nc.sync.dma_start(F2[:], f2_d.ap())

            f2_b = (
                F2[:]
                .rearrange("p (i b) -> p i b", b=2)
                .unsqueeze(1)
                .broadcast_to((128, _K, 128, 2))
            )

            for c in range(_NCHUNK):
                # 1. phase: U[p, jj, i, b] = f2_i * x_b(token jj)
                U = upool.tile([128, _CW], F32, tag="U")
                u4 = U[:].rearrange("p (jj i b) -> p jj i b", jj=_K, b=2)
                x_b = (
                    X[:, 3 * _K * c : 3 * _K * (c + 1)]
                    .rearrange("p (jj b) -> p jj b", b=3)[:, :, 0:2]
                    .unsqueeze(2)
                    .broadcast_to((128, _K, 128, 2))
                )
                nc.vector.tensor_tensor(u4, f2_b, x_b, ALU.mult)

                # 2. quarter turn on odd (cos) slots, in place
                u_odd = u4[:, :, :, 1:2]
                nc.scalar.activation(u_odd, u_odd, AF.Identity, bias=QB[:, 0:1])

                # 3. magic add: UM = fl(U + M) = k + M
                UM = umpool.tile([128, _CW], F32, tag="UM")
                if c % 16 == 0:
                    nc.vector.tensor_scalar(UM[:], U[:], _MAGIC, None, ALU.add)
                else:
                    nc.scalar.activation(UM[:], U[:], AF.Identity, bias=MB[:, 0:1])

                # 4. fused: R = (UM - M) - U = k - u = -r
                R = rpool.tile([128, _CW], F32, tag="R")
                nc.vector.scalar_tensor_tensor(
                    R[:], UM[:], _MAGIC, U[:], ALU.subtract, ALU.subtract
                )

                # 5. sin: O = sin(-2pi * R) = sin(2pi r)
                O = opool.tile([128, _CW], F32, tag="O")
                nc.scalar.activation(O[:], R[:], AF.Sin, scale=-_TWO_PI)

                # 6. store
                nc.sync.dma_start(out_view[:, c * _CW : (c + 1) * _CW], O[:])

    nc.compile()
    return nc


def _get_nc():
    if "nc" not in _cache:
        _cache["nc"] = _build()
    return _cache["nc"]


def _f2i_input():
    f2 = _freqs_half()
    f2i = np.empty(_D, np.float32)
    f2i[0::2] = f2
    f2i[1::2] = f2
    return np.tile(f2i[None, :], (128, 1))


def kernel(x: np.ndarray) -> np.ndarray:
    from concourse.bass_utils import run_bass_kernel_spmd

    nc = _get_nc()
    x = np.ascontiguousarray(np.asarray(x, dtype=np.float32))
    f2i = _f2i_input()
    shards = x.reshape(_NCORES, _NTOK, _C)
    in_maps = [{"x": shards[i], "f2i": f2i} for i in range(_NCORES)]
    res = run_bass_kernel_spmd(nc, in_maps, list(range(_NCORES)))
    out = np.stack([res.results[i]["out"] for i in range(_NCORES)])
    return out.reshape(_B, _S, _D)
